# revision 12
# baseline (speedup 1.0000x reference)
"""Block-causal attention kernel for trn2, sharded over 8 NeuronCores.

Sharding: device d handles batch b = d // 4 and heads hA = 2*(d%4),
hB = hA + 1.  Each device computes its two heads' attention plus its
partial output projection partialT[c, t] = sum_h Wo_h^T yT_h; the host
sums the 4 partials per batch and adds bo.

v2 design (vs baseline):
- bf16 everywhere on the matmul paths (same PE rate as f32r but halves
  DMA + SBUF and lifts the N>=256 restriction on small matmuls).
- AV flipped: y[q,65] = p[k,q]^T @ (v|1)[k,65] with p stationary -> 65
  moving cols per (k-chunk, q-chunk) pair instead of 512: ~2x less PE.
- v projected directly in [tok, hd] layout (x^T as stationary), no PE
  transposes for v.
- softmax denominator via the ones column of (v|1); normalization is a
  per-partition scalar multiply (no partition-broadcast DMA roundtrip).
- output projection packs both heads (contract dim 128) after a cheap
  bf16 transpose of y_norm.
- exp runs on Act only, head-paired into [128,2,512] PSUM tiles.
"""

import json

import numpy as np

import concourse.bass as bass
import concourse.mybir as mybir
import concourse.tile as tile
from concourse.bass_utils import run_bass_kernel_spmd
from concourse.masks import make_identity
from concourse.vector_clock import ScopedClock

F32 = mybir.dt.float32
BF16 = mybir.dt.bfloat16
F16 = mybir.dt.float16

VP, B, C, H, W = 8, 2, 512, 16, 16
NH = 8
HD = C // NH  # 64
HWD = H * W  # 256 = block size
T = VP * HWD  # 2048
NCORES = 8
SCALE = 1.0 / np.sqrt(HD)
EXP = mybir.ActivationFunctionType.Exp

# ---------------------------------------------------------------------------
# Container workarounds (walrus in this image rejects >1 sync wait/update per
# instruction; Tile's tail drain carries many).
# ---------------------------------------------------------------------------


def _split_syncs(bir_bytes: bytes) -> bytes:
    j = json.loads(bir_bytes)
    changed = False
    for fn in j.get("functions", []):
        for bb in fn.get("blocks", []):
            out = []
            for inst in bb.get("instructions", []):
                si = inst.get("sync_info")
                if not si:
                    out.append(inst)
                    continue
                waits = si.get("on_wait") or []
                upds = si.get("on_update") or []
                if len(waits) > 1:
                    for i, w in enumerate(waits[:-1]):
                        out.append(
                            {
                                "debug": inst.get("debug", 0),
                                "engine": inst["engine"],
                                "ins": [],
                                "name": f"{inst['name']}_sw{i}",
                                "opcode": "EventSemaphore",
                                "outs": [],
                                "sync_info": {"on_update": [], "on_wait": [w]},
                            }
                        )
                    si["on_wait"] = waits[-1:]
                    changed = True
                out.append(inst)
                if len(upds) > 1:
                    si["on_update"] = upds[:1]
                    for i, u in enumerate(upds[1:]):
                        out.append(
                            {
                                "debug": inst.get("debug", 0),
                                "engine": inst["engine"],
                                "ins": [],
                                "name": f"{inst['name']}_su{i}",
                                "opcode": "EventSemaphore",
                                "outs": [],
                                "sync_info": {"on_update": [u], "on_wait": []},
                            }
                        )
                    changed = True
            bb["instructions"] = out
    return json.dumps(j).encode() if changed else bir_bytes


_patched = False


def _install_patches():
    global _patched
    if _patched:
        return
    _patched = True

    import concourse.bass2jax as bass2jax
    from concourse.bass_utils import compile_bir_kernel as _real_compile

    def patched_compile(bir_json, tmpdir, neff_name="file.neff"):
        return _real_compile(_split_syncs(bir_json), tmpdir, neff_name=neff_name)

    bass2jax.compile_bir_kernel = patched_compile

    def _drain_and_barrier(self, tick_clock, wait_clock):
        nc = self.nc
        drain_inst = nc.sync.drain()
        wait_clock.add_sem_waits(
            drain_inst.ins, ScopedClock({None: tick_clock.global_clock})
        )
        si = drain_inst.ins.sync_info
        waits = list(si.on_wait or [])
        if len(waits) > 1:
            si.on_wait = waits[:1]
            for w in waits[1:]:
                d2 = nc.sync.drain()
                d2.ins.sync_info = mybir.SyncInfo(on_wait=[w], on_update=[])
        nc.all_engine_barrier()
        assert self.sems is not None
        popped = nc._tile_sem_poison_stack.pop()
        assert popped is self._sem_poison
        nc.clear_and_free_semaphores(list(self.sems.allocated().values()))
        nc.all_engine_barrier()

    tile.TileContext._drain_and_barrier = _drain_and_barrier


# ---------------------------------------------------------------------------
# Device program (SPMD - same program on all 8 cores, different data)
# ---------------------------------------------------------------------------


def _build_program():
    _install_patches()
    nc = bass.Bass("TRN2", target_bir_lowering=False, debug=False, num_devices=NCORES)

    xT = nc.dram_tensor("xT", [C, T], BF16, kind="ExternalInput")
    # w{q,k,v}: [p, cc, m] with c = cc*128 + p, m = head-cat output col
    wq = nc.dram_tensor("wq", [128, 4 * 128], BF16, kind="ExternalInput")
    wk = nc.dram_tensor("wk", [128, 4 * 128], BF16, kind="ExternalInput")
    wv = nc.dram_tensor("wv", [128, 4 * 128], BF16, kind="ExternalInput")
    # wo: [hd-cat(128), c] rows 0:64 head A, 64:128 head B
    wo = nc.dram_tensor("wo", [128, C], BF16, kind="ExternalInput")
    partialT = nc.dram_tensor("partialT", [C, T], BF16, kind="ExternalOutput")

    with tile.TileContext(nc) as tc:
        with (
            tc.tile_pool(name="persist", bufs=1) as pers,
            tc.tile_pool(name="qpool", bufs=2) as qpool,
            tc.tile_pool(name="ppool", bufs=3) as ppool,
            tc.tile_pool(name="ynp", bufs=3) as ynp,
            tc.tile_pool(name="ytp", bufs=2) as ytp,
            tc.tile_pool(name="rp", bufs=4) as rp,
            tc.tile_pool(name="stgp", bufs=3) as stgp,
            tc.tile_pool(name="stp", bufs=2, space="PSUM") as stp,
            tc.tile_pool(name="yp", bufs=1, space="PSUM") as yp,
            tc.tile_pool(name="pops", bufs=2, space="PSUM") as pops,
        ):
            # ---- persistent SBUF tiles
            xT_t = pers.tile([128, 4, T], BF16)
            wq_t = pers.tile([128, 4, 128], BF16)
            wk_t = pers.tile([128, 4, 128], BF16)
            wv_t = pers.tile([128, 4, 128], BF16)
            wo_t = pers.tile([128, 4, 128], BF16)
            kT_t = pers.tile([128, T], BF16)  # rows 0:64 head A, 64:128 head B
            vT_t = pers.tile([128, 16, 2, 65], BF16)  # [ktok, chunk, head, v|1]
            identB = pers.tile([128, 128], BF16)
            warm = pers.tile([128, 128], BF16)

            # ---- input DMA triggers: SP handles weights + spans 0/1,
            # Pool handles spans 2/3 (after identity setup).
            nc.sync.dma_start(
                out=wq_t[:], in_=wq.rearrange("p (c m) -> p c m", c=4)
            )
            for cc in range(3):
                nc.sync.dma_start(
                    out=xT_t[:, cc, 0:512], in_=xT[cc * 128 : (cc + 1) * 128, 0:512]
                )
            nc.scalar.dma_start(
                out=xT_t[:, 3, 0:512], in_=xT[384:512, 0:512]
            )
            nc.scalar.dma_start(
                out=wk_t[:], in_=wk.rearrange("p (c m) -> p c m", c=4)
            )
            nc.scalar.dma_start(
                out=wv_t[:], in_=wv.rearrange("p (c m) -> p c m", c=4)
            )
            nc.sync.dma_start(
                out=wo_t[:], in_=wo.rearrange("p (c m) -> p c m", c=4)
            )
            for cc in range(4):
                nc.sync.dma_start(
                    out=xT_t[:, cc, 512:1024], in_=xT[cc * 128 : (cc + 1) * 128, 512:1024]
                )

            make_identity(nc, identB)
            nc.vector.memset(warm[:], 0.0)
            nc.vector.memset(vT_t[:, :, :, 64:65], 1.0)

            for sp in range(2, 4):
                sl = slice(sp * 512, (sp + 1) * 512)
                for cc in range(4):
                    nc.gpsimd.dma_start(
                        out=xT_t[:, cc, sl], in_=xT[cc * 128 : (cc + 1) * 128, sl]
                    )

            # Act exp-table preload on a dummy tile (overlaps input DMA).
            nc.scalar.activation(warm[:, 0:1], identB[:, 0:1], EXP)

            # PE p-state warmup: dummy transposes while input DMA lands.
            for _ in range(24):
                wps = pops.tile([128, 128], BF16, tag="po")
                nc.tensor.transpose(wps[:], identB[:], identB[:])

            # -------------------------------------------------------------
            # building blocks
            # -------------------------------------------------------------
            qT_tiles = {}

            def qkv_pieces(sp):
                """PE+copy closures for span sp's q/k/v projection."""
                sl = slice(sp * 512, (sp + 1) * 512)
                pieces = []

                def q_piece():
                    psq = pops.tile([128, 512], F32, tag="po")
                    for cc in range(4):
                        nc.tensor.matmul(
                            psq[:],
                            wq_t[:, cc, :],
                            xT_t[:, cc, sl],
                            start=(cc == 0),
                            stop=(cc == 3),
                        )
                    qt = qpool.tile([128, 512], BF16, tag="qt")
                    nc.vector.tensor_copy(qt[:], psq[:])
                    qT_tiles[sp] = qt

                def k_piece():
                    psk = pops.tile([128, 512], F32, tag="po")
                    for cc in range(4):
                        nc.tensor.matmul(
                            psk[:],
                            wk_t[:, cc, :],
                            xT_t[:, cc, sl],
                            start=(cc == 0),
                            stop=(cc == 3),
                        )
                    nc.vector.tensor_copy(kT_t[:, sl], psk[:])

                def v_piece(tc):
                    psv = pops.tile([128, 2, 64], F32, tag="po")
                    for cc in range(4):
                        nc.tensor.matmul(
                            psv[:],
                            xT_t[:, cc, sp * 512 + tc * 128 : sp * 512 + (tc + 1) * 128],
                            wv_t[:, cc, :],
                            start=(cc == 0),
                            stop=(cc == 3),
                        )
                    j = 4 * sp + tc
                    nc.vector.tensor_copy(vT_t[:, j, :, 0:64], psv[:])

                pieces.append(q_piece)
                pieces.append(k_piece)
                for tc in range(4):
                    pieces.append(lambda tc=tc: v_piece(tc))
                return pieces

            yT_tiles = {}

            def wo_pieces(sp):
                """Output projection closures for span sp (reads yT_tiles[sp])."""
                sl = slice(sp * 512, (sp + 1) * 512)
                pieces = []
                for cc in range(4):

                    def piece(cc=cc):
                        yt = yT_tiles[sp]
                        po = pops.tile([128, 512], F32, tag="po")
                        nc.tensor.matmul(
                            po[:], wo_t[:, cc, :], yt[:], start=True, stop=True
                        )
                        stg = stgp.tile([128, 512], BF16, tag="stg")
                        nc.vector.tensor_copy(stg[:], po[:])
                        nc.sync.dma_start(
                            out=partialT[cc * 128 : (cc + 1) * 128, sl], in_=stg[:]
                        )

                    pieces.append(piece)
                return pieces

            def wo_half(sp, half, use_act):
                """Half-span output projection, pipelined for the tail."""
                q0 = sp * 512 + half * 256
                yt = yT_tiles[sp]
                for cc in range(4):
                    po = pops.tile([128, 256], F32, tag="po")
                    nc.tensor.matmul(
                        po[:],
                        wo_t[:, cc, :],
                        yt[:, half * 256 : half * 256 + 256],
                        start=True,
                        stop=True,
                    )
                    stg = stgp.tile([128, 256], BF16, tag="stg")
                    if use_act and cc % 2 == 0:
                        nc.scalar.activation(
                            stg[:], po[:], mybir.ActivationFunctionType.Copy
                        )
                        if True:
                            nc.scalar.dma_start(
                                out=partialT[cc * 128 : (cc + 1) * 128, q0 : q0 + 256],
                                in_=stg[:],
                            )
                        else:
                            nc.sync.dma_start(
                                out=partialT[cc * 128 : (cc + 1) * 128, q0 : q0 + 256],
                                in_=stg[:],
                            )
                    else:
                        nc.vector.tensor_copy(stg[:], po[:])
                        nc.sync.dma_start(
                            out=partialT[cc * 128 : (cc + 1) * 128, q0 : q0 + 256],
                            in_=stg[:],
                        )

            def emit_av(sp, j, p_t, off, Lg, yA, yB, njc):
                del Lg
                qcs = (2, 3) if off else (0, 1, 2, 3)
                for h, y in ((0, yA), (1, yB)):
                    for qc in qcs:
                        pcol = qc * 128 - off
                        stop = (j == 4 * sp + 1 and qc <= 1) or (
                            j == njc - 1 and qc >= 2
                        )
                        nc.tensor.matmul(
                            y[:, qc, 0:65],
                            p_t[:, h, pcol : pcol + 128],
                            vT_t[:, j, h, :],
                            start=(j == 0 and qc == 0),
                            stop=stop,
                        )

            def emit_norm_pair(sp, qclo, yA, yB, act_assist=False):
                rA = rp.tile([128, 2, 1], F32, tag="r")
                rB = rp.tile([128, 2, 1], F32, tag="r")
                nc.vector.reciprocal(rA[:], yA[:, qclo : qclo + 2, 64:65])
                nc.vector.reciprocal(rB[:], yB[:, qclo : qclo + 2, 64:65])
                for qc in (qclo, qclo + 1):
                    on_act = act_assist and qc % 2 == 1
                    yn = ynp.tile([128, 128], BF16, tag="yn")
                    if on_act:
                        nc.scalar.activation(
                            yn[:, 0:64],
                            yA[:, qc, 0:64],
                            mybir.ActivationFunctionType.Copy,
                            scale=rA[:, qc - qclo, :],
                        )
                        nc.scalar.activation(
                            yn[:, 64:128],
                            yB[:, qc, 0:64],
                            mybir.ActivationFunctionType.Copy,
                            scale=rB[:, qc - qclo, :],
                        )
                    else:
                        nc.vector.tensor_scalar_mul(
                            yn[:, 0:64], yA[:, qc, 0:64], rA[:, qc - qclo, :]
                        )
                        nc.vector.tensor_scalar_mul(
                            yn[:, 64:128], yB[:, qc, 0:64], rB[:, qc - qclo, :]
                        )
                    tp = pops.tile([128, 128], BF16, tag="po")
                    nc.tensor.transpose(tp[:], yn[:], identB[:])
                    yt = yT_tiles[sp]
                    if on_act:
                        nc.scalar.activation(
                            yt[:, qc * 128 : (qc + 1) * 128],
                            tp[:],
                            mybir.ActivationFunctionType.Copy,
                        )
                    else:
                        nc.vector.tensor_copy(yt[:, qc * 128 : (qc + 1) * 128], tp[:])


            # -------------------------------------------------------------
            # main pipeline
            # -------------------------------------------------------------
            for piece in qkv_pieces(0):
                piece()

            for sp in range(4):
                njc = 4 * sp + 4
                q0 = sp * 512
                fillers = []
                if sp < 3:
                    fillers += qkv_pieces(sp + 1)
                if sp > 0:
                    fillers += wo_pieces(sp - 1)
                yt_sp = ytp.tile([128, 512], BF16, tag="yt")
                yT_tiles[sp] = yt_sp
                yA = yp.tile([128, 4, 128], F32, tag="yA")
                yB = yp.tile([128, 4, 128], F32, tag="yB")

                p_tiles = {}
                drained = 0
                for j in range(njc):
                    off = 256 if j >= njc - 2 else 0
                    Lg = 512 - off
                    st = stp.tile([128, 2, 512], F32, tag="st")
                    for h in (0, 1):
                        nc.tensor.matmul(
                            st[:, h, 0:Lg],
                            kT_t[64 * h : 64 * h + 64, j * 128 : (j + 1) * 128],
                            qT_tiles[sp][64 * h : 64 * h + 64, off : off + Lg],
                            start=True,
                            stop=True,
                        )
                    p_t = ppool.tile([128, 2, 512], BF16, tag="p")
                    nc.scalar.activation(p_t[:, :, 0:Lg], st[:, :, 0:Lg], EXP)
                    p_tiles[j] = (p_t, off, Lg)
                    avdone = []
                    if j == 2:
                        avdone = [0, 1]
                    elif j >= 3:
                        avdone = [j - 1]
                    for ji in avdone:
                        pj, poff, pLg = p_tiles[ji]
                        emit_av(sp, ji, pj, poff, pLg, yA, yB, njc)
                        if ji == 4 * sp + 1:
                            emit_norm_pair(sp, 0, yA, yB)
                            if sp == 3:
                                wo_half(3, 0, use_act=False)
                    # drain fillers evenly, fully drained by j = njc-3
                    want = min(
                        len(fillers),
                        (len(fillers) * (j + 1) + (njc - 4)) // max(njc - 3, 1),
                    )
                    while drained < want:
                        fillers[drained]()
                        drained += 1
                pj, poff, pLg = p_tiles[njc - 1]
                emit_av(sp, njc - 1, pj, poff, pLg, yA, yB, njc)
                while drained < len(fillers):
                    fillers[drained]()
                    drained += 1
                emit_norm_pair(sp, 2, yA, yB, act_assist=(sp == 3))
                if sp == 3:
                    wo_half(3, 1, use_act=True)
    return nc


_NC_CACHE = None


def _get_program():
    global _NC_CACHE
    if _NC_CACHE is None:
        _NC_CACHE = _build_program()
    return _NC_CACHE


def kernel(x, Wqkv, bqkv, bo=None, Wo=None, **kw):
    import ml_dtypes

    if Wo is None:
        Wo = kw["Wo"]
    if bo is None:
        bo = kw["bo"]
    x = np.asarray(x, dtype=np.float32)
    Wqkv = np.asarray(Wqkv, dtype=np.float32)
    bqkv = np.asarray(bqkv, dtype=np.float32)
    Wo = np.asarray(Wo, dtype=np.float32)
    bo = np.asarray(bo, dtype=np.float32)
    assert np.all(bqkv == 0.0), "nonzero bqkv not supported by this kernel build"

    bf16 = ml_dtypes.bfloat16
    nc = _get_program()
    in_maps = []
    for d in range(NCORES):
        b = d // 4
        hA = 2 * (d % 4)
        hB = hA + 1
        # xT [C, T]: t = (v, h, w)
        xT = np.ascontiguousarray(
            x[:, b].transpose(1, 0, 2, 3).reshape(C, T).astype(bf16)
        )
        qcols = np.r_[hA * HD : (hA + 1) * HD, hB * HD : (hB + 1) * HD]

        def wlayout(wm):
            # [C, 128] -> [p, cc, m] -> [128, 512]
            return np.ascontiguousarray(
                wm.reshape(4, 128, 128).transpose(1, 0, 2).reshape(128, 512).astype(bf16)
            )

        wq_h = wlayout(Wqkv[:, qcols] * SCALE)
        wk_h = wlayout(Wqkv[:, C + qcols])
        wv_h = wlayout(Wqkv[:, 2 * C + qcols])
        wo_h = np.ascontiguousarray(
            np.concatenate(
                [Wo[hA * HD : (hA + 1) * HD], Wo[hB * HD : (hB + 1) * HD]], 0
            ).astype(bf16)
        )
        in_maps.append({"xT": xT, "wq": wq_h, "wk": wk_h, "wv": wv_h, "wo": wo_h})

    res = run_bass_kernel_spmd(nc, in_maps, core_ids=list(range(NCORES)))
    global _LAST_RES
    _LAST_RES = res

    out = np.empty((VP, B, C, H, W), dtype=np.float32)
    for b in range(B):
        acc = np.zeros((C, T), dtype=np.float32)
        for d in range(b * 4, b * 4 + 4):
            acc += res.results[d]["partialT"].astype(np.float32)
        acc += bo[:, None]
        out[:, b] = acc.reshape(C, VP, H, W).transpose(1, 0, 2, 3)
    return out


# revision 13
# speedup vs baseline: 1.0355x; 1.0355x over previous
"""Block-causal attention kernel for trn2, sharded over 8 NeuronCores.

Sharding: device d handles batch b = d // 4 and heads hA = 2*(d%4),
hB = hA + 1.  Each device computes its two heads' attention plus its
partial output projection partialT[c, t] = sum_h Wo_h^T yT_h; the host
sums the 4 partials per batch and adds bo.

v2 design (vs baseline):
- bf16 everywhere on the matmul paths (same PE rate as f32r but halves
  DMA + SBUF and lifts the N>=256 restriction on small matmuls).
- AV flipped: y[q,65] = p[k,q]^T @ (v|1)[k,65] with p stationary -> 65
  moving cols per (k-chunk, q-chunk) pair instead of 512: ~2x less PE.
- v projected directly in [tok, hd] layout (x^T as stationary), no PE
  transposes for v.
- softmax denominator via the ones column of (v|1); normalization is a
  per-partition scalar multiply (no partition-broadcast DMA roundtrip).
- output projection packs both heads (contract dim 128) after a cheap
  bf16 transpose of y_norm.
- exp runs on Act only, head-paired into [128,2,512] PSUM tiles.
"""

import json

import numpy as np

import concourse.bass as bass
import concourse.mybir as mybir
import concourse.tile as tile
from concourse.bass_utils import run_bass_kernel_spmd
from concourse.masks import make_identity
from concourse.vector_clock import ScopedClock

F32 = mybir.dt.float32
BF16 = mybir.dt.bfloat16
F16 = mybir.dt.float16

VP, B, C, H, W = 8, 2, 512, 16, 16
NH = 8
HD = C // NH  # 64
HWD = H * W  # 256 = block size
T = VP * HWD  # 2048
NCORES = 8
SCALE = 1.0 / np.sqrt(HD)
EXP = mybir.ActivationFunctionType.Exp

# ---------------------------------------------------------------------------
# Container workarounds (walrus in this image rejects >1 sync wait/update per
# instruction; Tile's tail drain carries many).
# ---------------------------------------------------------------------------


def _split_syncs(bir_bytes: bytes) -> bytes:
    j = json.loads(bir_bytes)
    changed = False
    for fn in j.get("functions", []):
        for bb in fn.get("blocks", []):
            out = []
            for inst in bb.get("instructions", []):
                si = inst.get("sync_info")
                if not si:
                    out.append(inst)
                    continue
                waits = si.get("on_wait") or []
                upds = si.get("on_update") or []
                if len(waits) > 1:
                    for i, w in enumerate(waits[:-1]):
                        out.append(
                            {
                                "debug": inst.get("debug", 0),
                                "engine": inst["engine"],
                                "ins": [],
                                "name": f"{inst['name']}_sw{i}",
                                "opcode": "EventSemaphore",
                                "outs": [],
                                "sync_info": {"on_update": [], "on_wait": [w]},
                            }
                        )
                    si["on_wait"] = waits[-1:]
                    changed = True
                out.append(inst)
                if len(upds) > 1:
                    si["on_update"] = upds[:1]
                    for i, u in enumerate(upds[1:]):
                        out.append(
                            {
                                "debug": inst.get("debug", 0),
                                "engine": inst["engine"],
                                "ins": [],
                                "name": f"{inst['name']}_su{i}",
                                "opcode": "EventSemaphore",
                                "outs": [],
                                "sync_info": {"on_update": [u], "on_wait": []},
                            }
                        )
                    changed = True
            bb["instructions"] = out
    return json.dumps(j).encode() if changed else bir_bytes


_patched = False


def _install_patches():
    global _patched
    if _patched:
        return
    _patched = True

    import concourse.bass2jax as bass2jax
    from concourse.bass_utils import compile_bir_kernel as _real_compile

    def patched_compile(bir_json, tmpdir, neff_name="file.neff"):
        return _real_compile(_split_syncs(bir_json), tmpdir, neff_name=neff_name)

    bass2jax.compile_bir_kernel = patched_compile

    def _drain_and_barrier(self, tick_clock, wait_clock):
        nc = self.nc
        drain_inst = nc.sync.drain()
        wait_clock.add_sem_waits(
            drain_inst.ins, ScopedClock({None: tick_clock.global_clock})
        )
        si = drain_inst.ins.sync_info
        waits = list(si.on_wait or [])
        if len(waits) > 1:
            si.on_wait = waits[:1]
            for w in waits[1:]:
                d2 = nc.sync.drain()
                d2.ins.sync_info = mybir.SyncInfo(on_wait=[w], on_update=[])
        nc.all_engine_barrier()
        assert self.sems is not None
        popped = nc._tile_sem_poison_stack.pop()
        assert popped is self._sem_poison
        nc.clear_and_free_semaphores(list(self.sems.allocated().values()))
        nc.all_engine_barrier()

    tile.TileContext._drain_and_barrier = _drain_and_barrier


# ---------------------------------------------------------------------------
# Device program (SPMD - same program on all 8 cores, different data)
# ---------------------------------------------------------------------------


def _build_program():
    _install_patches()
    nc = bass.Bass("TRN2", target_bir_lowering=False, debug=False, num_devices=NCORES)

    xT = nc.dram_tensor("xT", [C, T], BF16, kind="ExternalInput")
    # w{q,k,v}: [p, cc, m] with c = cc*128 + p, m = head-cat output col
    wq = nc.dram_tensor("wq", [128, 4 * 128], BF16, kind="ExternalInput")
    wk = nc.dram_tensor("wk", [128, 4 * 128], BF16, kind="ExternalInput")
    wv = nc.dram_tensor("wv", [128, 4 * 128], BF16, kind="ExternalInput")
    # wo: [hd-cat(128), c] rows 0:64 head A, 64:128 head B
    wo = nc.dram_tensor("wo", [128, C], BF16, kind="ExternalInput")
    partialT = nc.dram_tensor("partialT", [C, T], BF16, kind="ExternalOutput")

    with tile.TileContext(nc) as tc:
        with (
            tc.tile_pool(name="persist", bufs=1) as pers,
            tc.tile_pool(name="qpool", bufs=2) as qpool,
            tc.tile_pool(name="ppool", bufs=3) as ppool,
            tc.tile_pool(name="ynp", bufs=5) as ynp,
            tc.tile_pool(name="ytp", bufs=2) as ytp,
            tc.tile_pool(name="rp", bufs=6) as rp,
            tc.tile_pool(name="stgp", bufs=4) as stgp,
            tc.tile_pool(name="stp", bufs=2, space="PSUM") as stp,
            tc.tile_pool(name="yp", bufs=1, space="PSUM") as yp,
            tc.tile_pool(name="pops", bufs=2, space="PSUM") as pops,
        ):
            # ---- persistent SBUF tiles
            xT_t = pers.tile([128, 4, T], BF16)
            wq_t = pers.tile([128, 4, 128], BF16)
            wk_t = pers.tile([128, 4, 128], BF16)
            wv_t = pers.tile([128, 4, 128], BF16)
            wo_t = pers.tile([128, 4, 128], BF16)
            kT_t = pers.tile([128, T], BF16)  # rows 0:64 head A, 64:128 head B
            vT_t = pers.tile([128, 16, 2, 65], BF16)  # [ktok, chunk, head, v|1]
            identB = pers.tile([128, 128], BF16)
            warm = pers.tile([128, 128], BF16)

            # ---- input DMA triggers: SP handles weights + spans 0/1,
            # Pool handles spans 2/3 (after identity setup).
            nc.sync.dma_start(
                out=wq_t[:], in_=wq.rearrange("p (c m) -> p c m", c=4)
            )
            for cc in range(2):
                nc.sync.dma_start(
                    out=xT_t[:, cc, 0:512], in_=xT[cc * 128 : (cc + 1) * 128, 0:512]
                )
            nc.scalar.dma_start(
                out=xT_t[:, 2, 0:512], in_=xT[256:384, 0:512]
            )
            nc.scalar.dma_start(
                out=xT_t[:, 3, 0:512], in_=xT[384:512, 0:512]
            )
            nc.scalar.dma_start(
                out=wk_t[:], in_=wk.rearrange("p (c m) -> p c m", c=4)
            )
            nc.scalar.dma_start(
                out=wv_t[:], in_=wv.rearrange("p (c m) -> p c m", c=4)
            )
            nc.sync.dma_start(
                out=wo_t[:], in_=wo.rearrange("p (c m) -> p c m", c=4)
            )
            for cc in range(4):
                nc.sync.dma_start(
                    out=xT_t[:, cc, 512:1024], in_=xT[cc * 128 : (cc + 1) * 128, 512:1024]
                )

            make_identity(nc, identB)
            nc.vector.memset(warm[:], 0.0)
            nc.vector.memset(vT_t[:, :, :, 64:65], 1.0)

            for sp in range(2, 4):
                sl = slice(sp * 512, (sp + 1) * 512)
                for cc in range(4):
                    nc.gpsimd.dma_start(
                        out=xT_t[:, cc, sl], in_=xT[cc * 128 : (cc + 1) * 128, sl]
                    )

            # Act exp-table preload on a dummy tile (overlaps input DMA).
            nc.scalar.activation(warm[:, 0:1], identB[:, 0:1], EXP)

            # PE p-state warmup: dummy transposes while input DMA lands.
            for _ in range(24):
                wps = pops.tile([128, 128], BF16, tag="po")
                nc.tensor.transpose(wps[:], identB[:], identB[:])

            # -------------------------------------------------------------
            # building blocks
            # -------------------------------------------------------------
            qT_tiles = {}

            def qkv_pieces(sp):
                """PE+copy closures for span sp's q/k/v projection."""
                sl = slice(sp * 512, (sp + 1) * 512)
                pieces = []

                def q_piece():
                    psq = pops.tile([128, 512], F32, tag="po")
                    for cc in range(4):
                        nc.tensor.matmul(
                            psq[:],
                            wq_t[:, cc, :],
                            xT_t[:, cc, sl],
                            start=(cc == 0),
                            stop=(cc == 3),
                        )
                    qt = qpool.tile([128, 512], BF16, tag="qt")
                    nc.vector.tensor_copy(qt[:], psq[:])
                    qT_tiles[sp] = qt

                def k_piece():
                    psk = pops.tile([128, 512], F32, tag="po")
                    for cc in range(4):
                        nc.tensor.matmul(
                            psk[:],
                            wk_t[:, cc, :],
                            xT_t[:, cc, sl],
                            start=(cc == 0),
                            stop=(cc == 3),
                        )
                    nc.vector.tensor_copy(kT_t[:, sl], psk[:])

                def v_piece(tc):
                    psv = pops.tile([128, 2, 64], F32, tag="po")
                    for cc in range(4):
                        nc.tensor.matmul(
                            psv[:],
                            xT_t[:, cc, sp * 512 + tc * 128 : sp * 512 + (tc + 1) * 128],
                            wv_t[:, cc, :],
                            start=(cc == 0),
                            stop=(cc == 3),
                        )
                    j = 4 * sp + tc
                    nc.vector.tensor_copy(vT_t[:, j, :, 0:64], psv[:])

                pieces.append(q_piece)
                pieces.append(k_piece)
                for tc in range(4):
                    pieces.append(lambda tc=tc: v_piece(tc))
                return pieces

            yT_tiles = {}

            def wo_pieces(sp):
                """Output projection closures for span sp (reads yT_tiles[sp])."""
                sl = slice(sp * 512, (sp + 1) * 512)
                pieces = []
                for cc in range(4):

                    def piece(cc=cc):
                        yt = yT_tiles[sp]
                        po = pops.tile([128, 512], F32, tag="po")
                        nc.tensor.matmul(
                            po[:], wo_t[:, cc, :], yt[:], start=True, stop=True
                        )
                        stg = stgp.tile([128, 512], BF16, tag="stg")
                        nc.vector.tensor_copy(stg[:], po[:])
                        nc.sync.dma_start(
                            out=partialT[cc * 128 : (cc + 1) * 128, sl], in_=stg[:]
                        )

                    pieces.append(piece)
                return pieces

            def wo_half(sp, half, use_act):
                """Half-span output projection, pipelined for the tail."""
                q0 = sp * 512 + half * 256
                yt = yT_tiles[sp]
                for cc in range(4):
                    po = pops.tile([128, 256], F32, tag="po")
                    nc.tensor.matmul(
                        po[:],
                        wo_t[:, cc, :],
                        yt[:, half * 256 : half * 256 + 256],
                        start=True,
                        stop=True,
                    )
                    stg = stgp.tile([128, 256], BF16, tag="stg")
                    if use_act and cc % 2 == 0:
                        nc.scalar.activation(
                            stg[:], po[:], mybir.ActivationFunctionType.Copy
                        )
                        if True:
                            nc.scalar.dma_start(
                                out=partialT[cc * 128 : (cc + 1) * 128, q0 : q0 + 256],
                                in_=stg[:],
                            )
                        else:
                            nc.sync.dma_start(
                                out=partialT[cc * 128 : (cc + 1) * 128, q0 : q0 + 256],
                                in_=stg[:],
                            )
                    else:
                        nc.vector.tensor_copy(stg[:], po[:])
                        nc.sync.dma_start(
                            out=partialT[cc * 128 : (cc + 1) * 128, q0 : q0 + 256],
                            in_=stg[:],
                        )

            def emit_av(sp, j, p_t, off, Lg, yA, yB, njc):
                del Lg
                qcs = (2, 3) if off else (0, 1, 2, 3)
                for h, y in ((0, yA), (1, yB)):
                    for qc in qcs:
                        pcol = qc * 128 - off
                        stop = (j == 4 * sp + 1 and qc <= 1) or (
                            j == njc - 1 and qc >= 2
                        )
                        nc.tensor.matmul(
                            y[:, qc, 0:65],
                            p_t[:, h, pcol : pcol + 128],
                            vT_t[:, j, h, :],
                            start=(j == 0 and qc == 0),
                            stop=stop,
                        )

            def emit_norm_pair(sp, qclo, yA, yB, act_assist=False):
                rA = rp.tile([128, 2, 1], F32, tag="r")
                rB = rp.tile([128, 2, 1], F32, tag="r")
                nc.vector.reciprocal(rA[:], yA[:, qclo : qclo + 2, 64:65])
                nc.vector.reciprocal(rB[:], yB[:, qclo : qclo + 2, 64:65])
                if not act_assist:
                    yn_t = {}
                    for qc in (qclo, qclo + 1):
                        ynq = ynp.tile([128, 128], BF16, tag="yn")
                        yn_t[qc] = ynq
                        nc.vector.tensor_scalar_mul(
                            ynq[:, 0:64], yA[:, qc, 0:64], rA[:, qc - qclo, :]
                        )
                    for qc in (qclo, qclo + 1):
                        nc.vector.tensor_scalar_mul(
                            yn_t[qc][:, 64:128], yB[:, qc, 0:64], rB[:, qc - qclo, :]
                        )
                    for qc in (qclo, qclo + 1):
                        tp = pops.tile([128, 128], BF16, tag="po")
                        nc.tensor.transpose(tp[:], yn_t[qc][:], identB[:])
                        yt = yT_tiles[sp]
                        nc.vector.tensor_copy(
                            yt[:, qc * 128 : (qc + 1) * 128], tp[:]
                        )
                    return
                for qc in (qclo, qclo + 1):
                    on_act = act_assist and qc % 2 == 1
                    yn = ynp.tile([128, 128], BF16, tag="yn")
                    if on_act:
                        nc.scalar.activation(
                            yn[:, 0:64],
                            yA[:, qc, 0:64],
                            mybir.ActivationFunctionType.Copy,
                            scale=rA[:, qc - qclo, :],
                        )
                        nc.scalar.activation(
                            yn[:, 64:128],
                            yB[:, qc, 0:64],
                            mybir.ActivationFunctionType.Copy,
                            scale=rB[:, qc - qclo, :],
                        )
                    else:
                        nc.vector.tensor_scalar_mul(
                            yn[:, 0:64], yA[:, qc, 0:64], rA[:, qc - qclo, :]
                        )
                        nc.vector.tensor_scalar_mul(
                            yn[:, 64:128], yB[:, qc, 0:64], rB[:, qc - qclo, :]
                        )
                    tp = pops.tile([128, 128], BF16, tag="po")
                    nc.tensor.transpose(tp[:], yn[:], identB[:])
                    yt = yT_tiles[sp]
                    if on_act:
                        nc.scalar.activation(
                            yt[:, qc * 128 : (qc + 1) * 128],
                            tp[:],
                            mybir.ActivationFunctionType.Copy,
                        )
                    else:
                        nc.vector.tensor_copy(yt[:, qc * 128 : (qc + 1) * 128], tp[:])


            # -------------------------------------------------------------
            # main pipeline
            # -------------------------------------------------------------
            for piece in qkv_pieces(0):
                piece()

            for sp in range(4):
                njc = 4 * sp + 4
                q0 = sp * 512
                fillers = []
                if sp < 3:
                    fillers += qkv_pieces(sp + 1)
                if sp > 0:
                    fillers += wo_pieces(sp - 1)
                yt_sp = ytp.tile([128, 512], BF16, tag="yt")
                yT_tiles[sp] = yt_sp
                yA = yp.tile([128, 4, 128], F32, tag="yA")
                yB = yp.tile([128, 4, 128], F32, tag="yB")

                p_tiles = {}
                drained = 0
                for j in range(njc):
                    off = 256 if j >= njc - 2 else 0
                    Lg = 512 - off
                    st = stp.tile([128, 2, 512], F32, tag="st")
                    for h in (0, 1):
                        nc.tensor.matmul(
                            st[:, h, 0:Lg],
                            kT_t[64 * h : 64 * h + 64, j * 128 : (j + 1) * 128],
                            qT_tiles[sp][64 * h : 64 * h + 64, off : off + Lg],
                            start=True,
                            stop=True,
                        )
                    p_t = ppool.tile([128, 2, 512], BF16, tag="p")
                    if sp == 3 and j == njc - 1:
                        nc.scalar.activation(p_t[:, 0, 0:Lg], st[:, 0, 0:Lg], EXP)
                        nc.scalar.activation(p_t[:, 1, 0:Lg], st[:, 1, 0:Lg], EXP)
                    else:
                        nc.scalar.activation(p_t[:, :, 0:Lg], st[:, :, 0:Lg], EXP)
                    p_tiles[j] = (p_t, off, Lg)
                    avdone = []
                    if j == 2:
                        avdone = [0, 1]
                    elif j >= 3:
                        avdone = [j - 1]
                    for ji in avdone:
                        pj, poff, pLg = p_tiles[ji]
                        emit_av(sp, ji, pj, poff, pLg, yA, yB, njc)
                        if ji == 4 * sp + 1:
                            emit_norm_pair(sp, 0, yA, yB)
                            if sp == 3:
                                wo_half(3, 0, use_act=False)
                    # drain fillers evenly, fully drained by j = njc-3
                    want = min(
                        len(fillers),
                        (len(fillers) * (j + 1) + (njc - 4)) // max(njc - 3, 1),
                    )
                    while drained < want:
                        fillers[drained]()
                        drained += 1
                pj, poff, pLg = p_tiles[njc - 1]
                emit_av(sp, njc - 1, pj, poff, pLg, yA, yB, njc)
                while drained < len(fillers):
                    fillers[drained]()
                    drained += 1
                emit_norm_pair(sp, 2, yA, yB, act_assist=(sp == 3))
                if sp == 3:
                    wo_half(3, 1, use_act=True)
    return nc


_NC_CACHE = None


def _get_program():
    global _NC_CACHE
    if _NC_CACHE is None:
        _NC_CACHE = _build_program()
    return _NC_CACHE


def kernel(x, Wqkv, bqkv, bo=None, Wo=None, **kw):
    import ml_dtypes

    if Wo is None:
        Wo = kw["Wo"]
    if bo is None:
        bo = kw["bo"]
    x = np.asarray(x, dtype=np.float32)
    Wqkv = np.asarray(Wqkv, dtype=np.float32)
    bqkv = np.asarray(bqkv, dtype=np.float32)
    Wo = np.asarray(Wo, dtype=np.float32)
    bo = np.asarray(bo, dtype=np.float32)
    assert np.all(bqkv == 0.0), "nonzero bqkv not supported by this kernel build"

    bf16 = ml_dtypes.bfloat16
    nc = _get_program()
    in_maps = []
    for d in range(NCORES):
        b = d // 4
        hA = 2 * (d % 4)
        hB = hA + 1
        # xT [C, T]: t = (v, h, w)
        xT = np.ascontiguousarray(
            x[:, b].transpose(1, 0, 2, 3).reshape(C, T).astype(bf16)
        )
        qcols = np.r_[hA * HD : (hA + 1) * HD, hB * HD : (hB + 1) * HD]

        def wlayout(wm):
            # [C, 128] -> [p, cc, m] -> [128, 512]
            return np.ascontiguousarray(
                wm.reshape(4, 128, 128).transpose(1, 0, 2).reshape(128, 512).astype(bf16)
            )

        wq_h = wlayout(Wqkv[:, qcols] * SCALE)
        wk_h = wlayout(Wqkv[:, C + qcols])
        wv_h = wlayout(Wqkv[:, 2 * C + qcols])
        wo_h = np.ascontiguousarray(
            np.concatenate(
                [Wo[hA * HD : (hA + 1) * HD], Wo[hB * HD : (hB + 1) * HD]], 0
            ).astype(bf16)
        )
        in_maps.append({"xT": xT, "wq": wq_h, "wk": wk_h, "wv": wv_h, "wo": wo_h})

    res = run_bass_kernel_spmd(nc, in_maps, core_ids=list(range(NCORES)))
    global _LAST_RES
    _LAST_RES = res

    out = np.empty((VP, B, C, H, W), dtype=np.float32)
    for b in range(B):
        acc = np.zeros((C, T), dtype=np.float32)
        for d in range(b * 4, b * 4 + 4):
            acc += res.results[d]["partialT"].astype(np.float32)
        acc += bo[:, None]
        out[:, b] = acc.reshape(C, VP, H, W).transpose(1, 0, 2, 3)
    return out
